# revision 9
# baseline (speedup 1.0000x reference)
"""Trainium2 Bass kernel for the MANE multi-view SGNS embedding loss.

Strategy: data-parallel over the batch axis B across 8 NeuronCores.  The
embedding-row fetch (pure data movement, no arithmetic) happens in host_prep:
per-core contiguous bf16 streams of negative rows, positive rows and center
rows are built with numpy and handed to each core.  The device does all the
math: 5.3M 128-dim dot products (bf16 multiply + pairwise tree reduction on
the vector engine), log-sigmoid via Sigmoid+Ln on the scalar engine with
per-term accumulation, producing per-core partial sums [128, 2T] that the
host combines into the final scalar.

Negative streams are laid out k-major ([k-chunk][partition][k, j, d]) so the
multiply against a 2x-replicated center tile is a fully packed bf16
tensor_tensor (DVE 2x mode) instead of a broadcast multiply (1x).
"""

import numpy as np
import ml_dtypes

import concourse.bass as bass  # noqa: F401
import concourse.bacc as bacc
import concourse.tile as tile
from concourse import mybir
from concourse.bass_utils import run_bass_kernel_spmd

# ---------------------------------------------------------------- problem dims
V, N, D = 3, 200000, 128
B, K = 32768, 10
TOTAL = 65536
NCORES = 8
P = 128
T = 3 + 2 * V * (V - 1)  # 15 terms
KCH = 2                  # negatives per k-chunk
NKC = K // KCH           # k-chunks per term (5)

F32 = mybir.dt.float32
BF16 = mybir.dt.bfloat16

NPBF16 = ml_dtypes.bfloat16

# (j, i) pairs in reference order for cost2/cost3
PAIRS = [(j, i) for j in range(V) for i in range(V) if i != j]
# center view per term: cost1[i] -> i, cost2/3 (j,i) -> i
TERM_VIEW = [0, 1, 2] + [i for (_, i) in PAIRS] + [i for (_, i) in PAIRS]


def build_bass(bc, k, nchunk):
    """Per-core Tile program: stream pair-rows, dot, log-sigmoid, accumulate.

    bc: batch elems per core; k: negatives per positive; nchunk: unused
    (kept for cache-key compat), chunking is by k-pairs.
    """
    jb = bc // P              # batch columns per partition (32)

    nc = bacc.Bacc("TRN2", target_bir_lowering=False, debug=False,
                   enable_asserts=False)

    neg_s = nc.dram_tensor("neg_s", [T, NKC, P, KCH * jb * D], BF16,
                           kind="ExternalInput")
    pos_s = nc.dram_tensor("pos_s", [T, P, jb * D], BF16,
                           kind="ExternalInput")
    cen_s = nc.dram_tensor("cen_s", [V, P, jb * D], BF16,
                           kind="ExternalInput")
    acc_out = nc.dram_tensor("acc", [P, 2 * T], F32, kind="ExternalOutput")

    from contextlib import ExitStack
    with tile.TileContext(nc) as tc, ExitStack() as ctx:
        cen_pool = ctx.enter_context(tc.tile_pool(name="cen", bufs=1))
        neg_pool = ctx.enter_context(tc.tile_pool(name="neg", bufs=2))
        pos_pool = ctx.enter_context(tc.tile_pool(name="pos", bufs=2))
        scr_pool = ctx.enter_context(tc.tile_pool(name="scr", bufs=2))
        xn_pool = ctx.enter_context(tc.tile_pool(name="xn", bufs=2))
        out_pool = ctx.enter_context(tc.tile_pool(name="out", bufs=1))

        # resident centers, KCH-replicated: CEN2[v] [P, KCH*jb*D]
        CEN2 = []
        for v in range(V):
            c2 = cen_pool.tile([P, KCH * jb * D], BF16, tag=f"cen2_{v}")
            for r in range(KCH):
                nc.sync.dma_start(c2[:, r * jb * D:(r + 1) * jb * D],
                                  cen_s.ap()[v])
            CEN2.append(c2)

        ACC = out_pool.tile([P, 2 * T], F32)

        def tree_reduce(prod, m, width, tag, out_ap, eng=None):
            """Pairwise-sum last axis of [P, m, width] bf16 into out_ap [P,m]."""
            eng = eng or nc.vector
            cur = prod
            while width > 2:
                half = width // 2
                nxt = scr_pool.tile([P, m * half], BF16, tag=f"{tag}{half}")
                eng.tensor_tensor(
                    out=nxt[:].rearrange("p (m d) -> p m d", m=m),
                    in0=cur[:].rearrange("p (m d) -> p m d", m=m,
                                         d=width)[:, :, 0:half],
                    in1=cur[:].rearrange("p (m d) -> p m d", m=m,
                                         d=width)[:, :, half:width],
                    op=mybir.AluOpType.add)
                cur, width = nxt, half
            eng.tensor_tensor(
                out=out_ap.rearrange("p (m d) -> p m d", m=m),
                in0=cur[:].rearrange("p (m d) -> p m d", m=m, d=2)[:, :, 0:1],
                in1=cur[:].rearrange("p (m d) -> p m d", m=m, d=2)[:, :, 1:2],
                op=mybir.AluOpType.add)

        for t in range(T):
            iv = TERM_VIEW[t]
            XN = xn_pool.tile([P, K * jb], F32, tag="XN")
            for kc in range(NKC):
                NEG = neg_pool.tile([P, KCH * jb * D], BF16, tag="NEG")
                eng = nc.sync if (t * NKC + kc) % 2 == 0 else nc.scalar
                eng.dma_start(NEG[:], neg_s.ap()[t, kc])
                prod = neg_pool.tile([P, KCH * jb * D], BF16, tag="prod")
                nc.vector.tensor_tensor(out=prod[:], in0=NEG[:],
                                        in1=CEN2[iv][:],
                                        op=mybir.AluOpType.mult)
                m = KCH * jb
                tree_reduce(prod, m, D, "tn",
                            XN[:, kc * m:(kc + 1) * m])

            POSG = pos_pool.tile([P, jb * D], BF16, tag="posg")
            eng = nc.sync if t % 2 == 0 else nc.scalar
            eng.dma_start(POSG[:], pos_s.ap()[t])
            pprod = pos_pool.tile([P, jb * D], BF16, tag="pprod")
            nc.gpsimd.tensor_tensor(out=pprod[:], in0=POSG[:],
                                    in1=CEN2[iv][:, 0:jb * D],
                                    op=mybir.AluOpType.mult)
            XP = xn_pool.tile([P, jb], F32, tag="XP")
            tree_reduce(pprod, jb, D, "tp", XP[:], eng=nc.gpsimd)

            # log_sigmoid: neg sum uses ln(sigmoid(-x)), pos uses ln(sigmoid(x))
            sgn = xn_pool.tile([P, K * jb], F32, tag="sgn")
            nc.scalar.activation(
                out=sgn[:], in_=XN[:],
                func=mybir.ActivationFunctionType.Sigmoid, scale=-1.0)
            spn = xn_pool.tile([P, K * jb], F32, tag="spn")
            nc.scalar.activation(
                out=spn[:], in_=sgn[:],
                func=mybir.ActivationFunctionType.Ln,
                accum_out=ACC[:, t:t + 1])
            sgp = xn_pool.tile([P, jb], F32, tag="sgp")
            nc.scalar.activation(
                out=sgp[:], in_=XP[:],
                func=mybir.ActivationFunctionType.Sigmoid)
            spp = xn_pool.tile([P, jb], F32, tag="spp")
            nc.scalar.activation(
                out=spp[:], in_=sgp[:],
                func=mybir.ActivationFunctionType.Ln,
                accum_out=ACC[:, T + t:T + t + 1])

        nc.sync.dma_start(acc_out.ap(), ACC[:])

    nc.compile()
    return nc


_NC_CACHE = {}


def _get_nc(bc, k, nchunk):
    key = (bc, k, nchunk)
    if key not in _NC_CACHE:
        _NC_CACHE[key] = build_bass(bc, k, nchunk)
    return _NC_CACHE[key]


def host_prep(count, shuffle_indices, nodes_idx, neigh_idx,
              neg_idx1, neg_idx2, neg_idx3, node_W, neigh_W,
              n_cores=NCORES, nchunk=4, b=B):
    """Build per-core bf16 row streams (negatives / positives / centers).

    Batch-elem mapping within a core: b_local = p*jb + j maps to partition p,
    column j.  Negative stream is k-major: [T, NKC, P, (kk, j, d)].
    """
    c0 = int(count)
    sh = np.asarray(shuffle_indices)[:, c0:c0 + b].astype(np.int64)
    nodes_sel = np.take_along_axis(np.asarray(nodes_idx).astype(np.int64), sh, axis=1)
    neigh_sel = np.take_along_axis(np.asarray(neigh_idx).astype(np.int64), sh, axis=1)
    neg1 = np.asarray(neg_idx1).astype(np.int64)[:, :b]
    neg2 = np.asarray(neg_idx2).astype(np.int64)[:, :, :b]
    neg3 = np.asarray(neg_idx3).astype(np.int64)[:, :, :b]

    node_Wb = np.asarray(node_W).astype(NPBF16)      # [V, N, D]
    neigh_Wb = np.asarray(neigh_W).astype(NPBF16)
    n = node_Wb.shape[1]
    W_all = np.concatenate(
        [node_Wb.reshape(V * n, D), neigh_Wb.reshape(V * n, D)], axis=0)

    # per-term global row indices, identical to the reference formulation
    pos_list, neg_list = [], []
    for i in range(V):
        pos_list.append(neigh_sel[i] + (V + i) * n)
        neg_list.append(neg1[i] + (V + i) * n)
    for (j, i) in PAIRS:
        pos_list.append(nodes_sel[i] + j * n)
        neg_list.append(neg2[j, i] + j * n)
    for (j, i) in PAIRS:
        pos_list.append(neigh_sel[i] + (V + j) * n)
        neg_list.append(neg3[j, i] + (V + j) * n)
    pos_all = np.stack(pos_list)                     # [T, b]
    neg_all = np.stack(neg_list)                     # [T, b, K]
    cen_all = nodes_sel + (np.arange(V) * n)[:, None]  # [V, b]

    bc = b // n_cores
    jb = bc // P
    k = neg_all.shape[-1]

    in_maps = []
    for core in range(n_cores):
        sl = slice(core * bc, (core + 1) * bc)
        # negatives: [T, bc, K] -> k-major [T, NKC, P, KCH, jb, D]
        nidx = neg_all[:, sl].reshape(T, P, jb, NKC, KCH)
        nidx = nidx.transpose(0, 3, 1, 4, 2)         # [T, NKC, P, KCH, jb]
        neg_rows = W_all[nidx.ravel()].reshape(T, NKC, P, KCH * jb * D)
        # positives: [T, bc] -> [T, P, jb*D]
        pidx = pos_all[:, sl].reshape(T, P, jb)
        pos_rows = W_all[pidx.ravel()].reshape(T, P, jb * D)
        # centers: [V, bc] -> [V, P, jb*D]
        cidx = cen_all[:, sl].reshape(V, P, jb)
        cen_rows = W_all[cidx.ravel()].reshape(V, P, jb * D)
        in_maps.append({
            "neg_s": neg_rows,
            "pos_s": pos_rows,
            "cen_s": cen_rows,
        })
    return in_maps


def host_combine(acc_list, hyp1, hyp2, b=B):
    """acc_list: per-core [P, 2T] partial sums -> final scalar."""
    s = np.zeros(T, dtype=np.float64)
    for a in acc_list:
        a = np.asarray(a, dtype=np.float64).sum(axis=0)
        s += a[:T] + a[T:2 * T]
    term_val = s / b
    cost1 = term_val[0:3].mean()
    cost2 = float(np.asarray(hyp1).reshape(-1)[0]) * term_val[3:9].sum() / 6.0
    cost3 = float(np.asarray(hyp2).reshape(-1)[0]) * term_val[9:15].sum() / 6.0
    return np.array(-(cost1 + cost2 + cost3) / 3.0, dtype=np.float32)


def kernel(count, shuffle_indices, nodes_idx, neigh_idx,
           neg_idx1, neg_idx2, neg_idx3, node_W, neigh_W, hyp1, hyp2):
    in_maps = host_prep(count, shuffle_indices, nodes_idx, neigh_idx,
                        neg_idx1, neg_idx2, neg_idx3, node_W, neigh_W)
    nc = _get_nc(B // NCORES, K, 4)
    res = run_bass_kernel_spmd(nc, in_maps, core_ids=list(range(NCORES)))
    acc_list = [r["acc"] for r in res.results]
    return host_combine(acc_list, hyp1, hyp2)
